# revision 10
# baseline (speedup 1.0000x reference)
"""Trainium2 Bass kernel for the fused candidate-attention module (fp8 DRSW).

Computation (reference, fp32):
    ds[n,m,l]  = self_delta[n,m,l,0] + self_delta[n,m,l,1]
    out[n,l]   = sum_m value_w[m] * ds[n,m,l] * (emb[1+l,:] . self_attn[n,m,:])

Sharding: data-parallel over N across 8 cores, B = 4 batches/core, full L.

Quantization scheme (fits the 2e-2 rel-err budget at ~1.3%):
    delta is shipped as fp8e4 of (delta - 0.5); the mul-phase matmul uses
    DoubleRowSwInterleave (DRSW) so BOTH k-planes contract in ONE pass at
    0.5 cyc/row with a free weight load (~112 ns per 512-chunk vs 766 ns
    for the fp16 2-matmul baseline).  The +0.5+0.5 offset is restored
    exactly via an extra contraction row: partition CROW of the moving
    tile is a host-packed constant 1.0, and the matching weight slots hold
    (uh, ul) = an fp8 hi/lo split of u[d] = sum_m w8[m,d], computed on
    device from the *quantized* weights, so the constant part of ds
    carries no w-quantization error.

DRSW layouts (decoded by probe_drsw3.py, exact):
    weights tile [P, 2, 128] flat bytes: byte (p, 2c+j) = w_j[p, 127-c]
      (pairs interleaved, d reversed; host pre-reverses attnT so the
       reversal cancels everywhere on device)
    moving AP [P, 2, 512]: dim1 = the two k-planes as strided blocks
    out[d, f] = sum_p w_0[p,d]*m_0[p,f] + w_1[p,d]*m_1[p,f]

Per-core pipeline per batch: NSPLIT sub-DMAs of [P, 2, LH] fp8 (sync ring);
w8 prep on DVE (vw*attnT into both j slots); u row via a 1-col matmul at
tile_position (0, CROW); 16 DRSW chunk matmuls into PSUM; evict/mul split
between ACT(+DVE) and direct-DVE per K_MULDVE; rows via ones32 col-group
matmuls packed 4/bank, deferred one batch; fp16 out staging.

Env knobs: K_NSPLIT, K_MULDVE (direct-DVE chunks per 16), K_DDBUFS,
K_GBUFS, K_RING, K_ROWDVE (row evictions on DVE instead of ACT),
K_WLO (second w_lo DRSW pass, default 1; halves the w-quant error),
KERNEL_STAGE=dma|mm|mul|full, KERNEL_LOOP=<R>, K_NBATCH.
"""

import os
from contextlib import ExitStack

import numpy as np
import ml_dtypes

import concourse.bacc as bacc
import concourse.bass as bass
import concourse.mybir as mybir
from concourse.bass_utils import run_bass_kernel_spmd
from concourse.tile import TileContext

N, M, L, K, D = 32, 100, 8192, 2, 128
NCORES = 8
B = N // NCORES
MMF = 512
NCHUNK = L // MMF  # 16
NBANKQ = 4
NQ = NCHUNK // NBANKQ

P = 101          # 100 data rows + 1 constant row
CROW = 100       # partition of the constant/offset row
# fp16 value whose two bytes are both e4m3 1.0 (0x38): the const row is
# memset once per dd buffer through the fp16 view
CONST16 = 0.52734375

F32 = mybir.dt.float32
F16 = mybir.dt.float16
F8 = mybir.dt.float8e4
E4 = ml_dtypes.float8_e4m3
DRSW = mybir.MatmulPerfMode.DoubleRowSwInterleave

NSPLIT = int(os.environ.get("K_NSPLIT", "4"))
LH = L // NSPLIT
CPS = NCHUNK // NSPLIT  # chunks per split
MULDVE = int(os.environ.get("K_MULDVE", "4"))
DDBUFS = int(os.environ.get("K_DDBUFS", "2"))
GBUFS = int(os.environ.get("K_GBUFS", "3"))
RING = os.environ.get("K_RING", "1")
ROWDVE = os.environ.get("K_ROWDVE", "0") == "1"
ROWALT = os.environ.get("K_ROWALT", "1") == "1"  # alternate row evict engine
IROWS = os.environ.get("K_IROWS", "1") == "1"  # interleave rows into mul phase
ROWBUFS = int(os.environ.get("K_ROWBUFS", "4"))
UBUFS = int(os.environ.get("K_UBUFS", "1"))
WLO = os.environ.get("K_WLO", "1") == "1"
LOOP_R = int(os.environ.get("KERNEL_LOOP", "1"))
STAGE = os.environ.get("KERNEL_STAGE", "full")
NBATCH = int(os.environ.get("K_NBATCH", str(B)))

# spread direct-DVE chunks through the 16 so ACT/DVE interleave
_DIRECT = set()
if MULDVE > 0:
    _DIRECT = {int(round(i * NCHUNK / MULDVE)) % NCHUNK for i in range(MULDVE)}


def _build_nc() -> bass.Bass:
    nc = bacc.Bacc()

    # dd8 bytes are fp8e4, but DMA'd as fp16 pairs: the 1-byte-dtype DMA
    # path is ~10x slower on this toolchain; the matmul reads a bitcast view.
    dd8 = nc.declare_dram_parameter("dd8", [B, NSPLIT, M, 2, LH // 2], F16, isOutput=False)
    attnT = nc.declare_dram_parameter("attnT", [P, B * D], F16, isOutput=False)
    embT = nc.declare_dram_parameter("embT", [D, L], F16, isOutput=False)
    vw = nc.declare_dram_parameter("vw", [P, 1], F32, isOutput=False)
    # outp[j, (b*NQ + q)*MMF + f] = out[b, (q*NBANKQ + j)*MMF + f]
    outp = nc.declare_dram_parameter("outp", [NBANKQ, B * NQ * MMF], F16, isOutput=True)

    with TileContext(nc) as tc, ExitStack() as ctx:
        const = ctx.enter_context(tc.tile_pool(name="const", bufs=1))

        vw_sb = const.tile([P, 1], F32)
        nc.scalar.dma_start(out=vw_sb[:], in_=vw[:])
        vw16_sb = const.tile([P, 5], F16)
        for i in range(5):
            nc.scalar.copy(vw16_sb[:, i : i + 1], vw_sb[:])
        attnT_sb = const.tile([P, B * D], F16)
        nc.scalar.dma_start(out=attnT_sb[:], in_=attnT[:])
        embT_sb = const.tile([D, L], F16)
        nc.scalar.dma_start(out=embT_sb[:], in_=embT[:])
        ones32 = const.tile([D, 32], F16)
        nc.vector.memset(ones32[:], 1.0)
        ones101 = const.tile([P, 1], F8)
        nc.vector.memset(ones101[:], 1.0)
        out_sb = const.tile([D, B * NQ * MMF], F16)

        dd_pool = ctx.enter_context(tc.tile_pool(name="dd", bufs=DDBUFS * NSPLIT))
        # pre-write the constant row (two e4m3 1.0 bytes per fp16 slot) in
        # every rotating buffer; in-loop DMAs only touch rows 0..M-1 and the
        # loop allocates a multiple of the buf count per iteration, so the
        # rotation stays aligned and these rows persist.
        # engine APs need base partition % 32 == 0: memset rows 96..100 (the
        # in-loop DMAs overwrite 96..99 with data; row 100 = the const row)
        for _ in range(DDBUFS * NSPLIT):
            t = dd_pool.tile([P, 2, LH // 2], F16, tag="dd")
            nc.vector.memset(t[96:P, :, :], CONST16)
        w_pool = ctx.enter_context(tc.tile_pool(name="w", bufs=2))
        u_psum = ctx.enter_context(tc.tile_pool(name="u", bufs=UBUFS, space="PSUM"))
        g_psum = ctx.enter_context(tc.tile_pool(name="g", bufs=GBUFS, space="PSUM"))
        gs_pool = ctx.enter_context(tc.tile_pool(name="gs", bufs=3))
        prod_pool = ctx.enter_context(tc.tile_pool(name="prod", bufs=2))
        row_psum = ctx.enter_context(tc.tile_pool(name="row", bufs=ROWBUFS, space="PSUM"))

        def emit_rows_q(b, prod, q):
            row = row_psum.tile([D, MMF], F32, tag="row")
            for j in range(NBANKQ):
                h = q * NBANKQ + j
                lsl = slice(h * MMF, (h + 1) * MMF)
                nc.tensor.matmul(
                    row[32 * j : 32 * j + 32, :],
                    lhsT=ones32[:],
                    rhs=prod[:, lsl],
                    start=True,
                    stop=True,
                    tile_position=(0, 32 * j),
                )
            on_dve = (q % 2 == 0) if ROWALT else ROWDVE
            osl = out_sb[:, (b * NQ + q) * MMF : (b * NQ + q + 1) * MMF]
            if on_dve:
                nc.vector.tensor_scalar(
                    out=osl, in0=row[:], scalar1=0.0, scalar2=None,
                    op0=mybir.AluOpType.add,
                )
            else:
                nc.scalar.copy(osl, row[:])

        def emit_rows(b, prod):
            for q in range(NQ):
                emit_rows_q(b, prod, q)

        loop_ctx = tc.For_i(0, LOOP_R, 1) if LOOP_R > 1 else None
        if loop_ctx is not None:
            ctx.enter_context(loop_ctx)
        pending = []
        for b in range(NBATCH):
            eng = nc.gpsimd if RING == "g" else nc.sync
            subs = []
            for s in range(NSPLIT):
                t = dd_pool.tile([P, 2, LH // 2], F16, tag="dd")
                eng.dma_start(out=t[0:M], in_=dd8[b, s])
                subs.append(t[:].bitcast(F8))
            if STAGE == "dma":
                continue

            # weights: w16 = vw*attnT (fp16, d-reversed); hi slots = e4m3(w16);
            # lo slots = e4m3(w16 - hi).  Row CROW: w16 = 0 (vw[CROW]=0) so the
            # lo tile's CROW slots are 0; the hi tile's CROW slots get (uh, ul),
            # an fp8 hi/lo split of the EXACT u = sum_m vw*attn (fp16 matmul),
            # so the const part of ds carries no w-quantization error.
            asl = attnT_sb[:, b * D : (b + 1) * D]
            w16 = w_pool.tile([P, D], F16, tag="w16")
            nc.vector.tensor_scalar(
                out=w16[0:M], in0=asl[0:M], scalar1=vw_sb[0:M, 0:1],
                scalar2=None, op0=mybir.AluOpType.mult,
            )
            # exact u = sum_m vw*attn (fp16 matmul, fp32 accumulate),
            # replicated over psum partitions 96..100 (col base must be %32)
            u_t = u_psum.tile([D, MMF], F32)
            nc.tensor.matmul(
                u_t[96:P, 0:D],
                lhsT=vw16_sb[:],
                rhs=asl,
                start=True,
                stop=True,
                tile_position=(0, 96),
            )
            w8 = w_pool.tile([P, 256], F8, tag="w8")
            w8mm = w8[:].rearrange("p (a b) -> p a b", a=2)
            wv = w8[:].rearrange("p (c j) -> p c j", j=2)
            # const-row weights (uh, ul) land on rows 96..100 FIRST; the data
            # writes below then overwrite rows 96..99, leaving row 100 = (uh, ul)
            nc.scalar.copy(wv[96:P, :, 0], u_t[96:P, 0:D])
            nc.vector.tensor_tensor(
                out=wv[96:P, :, 1],
                in0=u_t[96:P, 0:D],
                in1=wv[96:P, :, 0],
                op=mybir.AluOpType.subtract,
            )
            for j in range(2):
                nc.vector.tensor_scalar(
                    out=wv[0:M, :, j], in0=asl[0:M], scalar1=vw_sb[0:M, 0:1],
                    scalar2=None, op0=mybir.AluOpType.mult,
                )
            if WLO:
                wlo8 = w_pool.tile([P, 256], F8, tag="wlo8")
                wlo8mm = wlo8[:].rearrange("p (a b) -> p a b", a=2)
                wlov = wlo8[:].rearrange("p (c j) -> p c j", j=2)
                nc.vector.memset(wlo8[96:P, :], 0.0)
                for j in range(2):
                    nc.vector.tensor_tensor(
                        out=wlov[0:M, :, j], in0=w16[0:M], in1=wv[0:M, :, 0],
                        op=mybir.AluOpType.subtract,
                    )

            prod = prod_pool.tile([D, L], F16)
            for h in range(NCHUNK):
                lsl = slice(h * MMF, (h + 1) * MMF)
                sub = subs[h // CPS]
                c = h % CPS
                g = g_psum.tile([D, MMF], F32)
                rsl = sub[:, :, c * MMF : (c + 1) * MMF]
                nc.tensor.matmul(
                    g[:], lhsT=w8mm, rhs=rsl,
                    start=True, stop=not WLO, perf_mode=DRSW,
                )
                if WLO:
                    nc.tensor.matmul(
                        g[:], lhsT=wlo8mm, rhs=rsl,
                        start=False, stop=True, perf_mode=DRSW,
                    )
                if STAGE == "mm":
                    continue
                if h in _DIRECT:
                    nc.vector.tensor_mul(prod[:, lsl], g[:], embT_sb[:, lsl])
                else:
                    gs = gs_pool.tile([D, MMF], F16)
                    nc.scalar.copy(gs[:], g[:])
                    nc.vector.tensor_mul(prod[:, lsl], gs[:], embT_sb[:, lsl])
                if (
                    STAGE == "full"
                    and IROWS
                    and pending
                    and (h + 1) % NBANKQ == 0
                ):
                    # previous batch's row-quarter rides the PE queue right
                    # after this chunk: deps long satisfied, keeps PE warm
                    pb, pprod = pending[0]
                    emit_rows_q(pb, pprod, h // NBANKQ)
            if STAGE in ("mm", "mul"):
                continue

            # defer rows one batch so deps precede them in every engine FIFO
            if IROWS and pending:
                pending.pop(0)
            pending.append((b, prod))
            if not IROWS and len(pending) > 1:
                emit_rows(*pending.pop(0))
        for bp in pending:
            emit_rows(*bp)

        if STAGE == "full":
            for j in range(NBANKQ):
                nc.scalar.dma_start(
                    out=outp[j], in_=out_sb[32 * j : 32 * j + 1, :]
                )

    nc.compile()
    return nc


_NC_CACHE: dict[str, bass.Bass] = {}


def _get_nc() -> bass.Bass:
    key = (
        f"{MULDVE}:{DDBUFS}:{GBUFS}:{LOOP_R}:{STAGE}:"
        f"{RING}:{NBATCH}:{NSPLIT}:{ROWDVE}:{WLO}:{ROWALT}:{IROWS}:{ROWBUFS}:{UBUFS}"
    )
    if key not in _NC_CACHE:
        _NC_CACHE[key] = _build_nc()
    return _NC_CACHE[key]


def make_in_maps(self_attn, self_delta, emb_table, value_w):
    self_attn = np.asarray(self_attn, dtype=np.float32)
    self_delta = np.asarray(self_delta, dtype=np.float32)
    emb_table = np.asarray(emb_table, dtype=np.float32)
    value_w = np.asarray(value_w, dtype=np.float32).reshape(M)
    assert self_attn.shape == (N, M, D), self_attn.shape
    assert self_delta.shape == (N, M, L, K), self_delta.shape
    assert emb_table.shape == (L + 1, D), emb_table.shape

    # fp8 of (delta - 0.5), laid out [N, NSPLIT, M, 2, LH] (data rows only;
    # the const row lives in SBUF, pre-memset per buffer)
    r8 = (self_delta - np.float32(0.5)).astype(E4)  # [N, M, L, K]
    dd8 = np.ascontiguousarray(
        r8.reshape(N, M, NSPLIT, LH, K).transpose(0, 2, 1, 4, 3)
    )  # [N, NSPLIT, M, K, LH]

    # attnT [P, B*D] fp16, d-reversed, row CROW zero; vw [P,1] with CROW=0
    a = self_attn[:, :, ::-1]  # [N, M, D] reversed d
    attnT_full = np.zeros((N, P, D), dtype=np.float16)
    attnT_full[:, :M] = a
    vw_p = np.zeros((P, 1), dtype=np.float32)
    vw_p[:M, 0] = value_w

    embT16 = np.ascontiguousarray(emb_table[1:].T.astype(np.float16))

    in_maps = []
    for c in range(NCORES):
        nsl = slice(c * B, (c + 1) * B)
        in_maps.append({
            "dd8": np.ascontiguousarray(dd8[nsl]).view(np.float16),
            "attnT": np.ascontiguousarray(
                attnT_full[nsl].transpose(1, 0, 2)
            ).reshape(P, B * D),
            "embT": embT16,
            "vw": vw_p,
        })
    return in_maps


def decode_out(res_outp: np.ndarray) -> np.ndarray:
    """[NBANKQ, B*NQ*MMF] staged fp16 layout -> [B, L] fp32 per core."""
    return (
        res_outp.reshape(NBANKQ, B, NQ, MMF)
        .transpose(1, 2, 0, 3)
        .reshape(B, L)
        .astype(np.float32)
    )


def kernel(self_attn, self_delta, emb_table, value_w, traj_len=None, loc_max=None):
    """Full inputs in, full output out.  traj_len is unused by the reference."""
    if loc_max is not None:
        assert int(loc_max) == L, loc_max
    in_maps = make_in_maps(self_attn, self_delta, emb_table, value_w)

    nc = _get_nc()
    try:
        res = run_bass_kernel_spmd(nc, in_maps, list(range(NCORES)))
    except Exception:
        res = run_bass_kernel_spmd(nc, in_maps, list(range(NCORES)))
    out = np.concatenate(
        [decode_out(res.results[c]["outp"]) for c in range(NCORES)], axis=0
    )
    return out


# revision 11
# speedup vs baseline: 1.0325x; 1.0325x over previous
"""Trainium2 Bass kernel for the fused candidate-attention module (fp8 DRSW).

Computation (reference, fp32):
    ds[n,m,l]  = self_delta[n,m,l,0] + self_delta[n,m,l,1]
    out[n,l]   = sum_m value_w[m] * ds[n,m,l] * (emb[1+l,:] . self_attn[n,m,:])

Sharding: data-parallel over N across 8 cores, B = 4 batches/core, full L.

Quantization scheme (fits the 2e-2 rel-err budget at ~1.3%):
    delta is shipped as fp8e4 of (delta - 0.5); the mul-phase matmul uses
    DoubleRowSwInterleave (DRSW) so BOTH k-planes contract in ONE pass at
    0.5 cyc/row with a free weight load (~112 ns per 512-chunk vs 766 ns
    for the fp16 2-matmul baseline).  The +0.5+0.5 offset is restored
    exactly via an extra contraction row: partition CROW of the moving
    tile is a host-packed constant 1.0, and the matching weight slots hold
    (uh, ul) = an fp8 hi/lo split of u[d] = sum_m w8[m,d], computed on
    device from the *quantized* weights, so the constant part of ds
    carries no w-quantization error.

DRSW layouts (decoded by probe_drsw3.py, exact):
    weights tile [P, 2, 128] flat bytes: byte (p, 2c+j) = w_j[p, 127-c]
      (pairs interleaved, d reversed; host pre-reverses attnT so the
       reversal cancels everywhere on device)
    moving AP [P, 2, 512]: dim1 = the two k-planes as strided blocks
    out[d, f] = sum_p w_0[p,d]*m_0[p,f] + w_1[p,d]*m_1[p,f]

Per-core pipeline per batch: NSPLIT sub-DMAs of [P, 2, LH] fp8 (sync ring);
w8 prep on DVE (vw*attnT into both j slots); u row via a 1-col matmul at
tile_position (0, CROW); 16 DRSW chunk matmuls into PSUM; evict/mul split
between ACT(+DVE) and direct-DVE per K_MULDVE; rows via ones32 col-group
matmuls packed 4/bank, deferred one batch; fp16 out staging.

Env knobs: K_NSPLIT, K_MULDVE (direct-DVE chunks per 16), K_DDBUFS,
K_GBUFS, K_RING, K_ROWDVE (row evictions on DVE instead of ACT),
K_WLO (second w_lo DRSW pass, default 1; halves the w-quant error),
KERNEL_STAGE=dma|mm|mul|full, KERNEL_LOOP=<R>, K_NBATCH.
"""

import os
from contextlib import ExitStack

import numpy as np
import ml_dtypes

import concourse.bacc as bacc
import concourse.bass as bass
import concourse.mybir as mybir
from concourse.bass_utils import run_bass_kernel_spmd
from concourse.tile import TileContext

N, M, L, K, D = 32, 100, 8192, 2, 128
NCORES = 8
B = N // NCORES
MMF = 512
NCHUNK = L // MMF  # 16
NBANKQ = 4
NQ = NCHUNK // NBANKQ

P = 101          # 100 data rows + 1 constant row
CROW = 100       # partition of the constant/offset row
# fp16 value whose two bytes are both e4m3 1.0 (0x38): the const row is
# memset once per dd buffer through the fp16 view
CONST16 = 0.52734375

F32 = mybir.dt.float32
F16 = mybir.dt.float16
F8 = mybir.dt.float8e4
E4 = ml_dtypes.float8_e4m3
DRSW = mybir.MatmulPerfMode.DoubleRowSwInterleave

NSPLIT = int(os.environ.get("K_NSPLIT", "4"))
LH = L // NSPLIT
CPS = NCHUNK // NSPLIT  # chunks per split
MULDVE = int(os.environ.get("K_MULDVE", "4"))
DDBUFS = int(os.environ.get("K_DDBUFS", "2"))
GBUFS = int(os.environ.get("K_GBUFS", "3"))
RING = os.environ.get("K_RING", "1")
ROWDVE = os.environ.get("K_ROWDVE", "0") == "1"
ROWALT = os.environ.get("K_ROWALT", "1") == "1"  # alternate row evict engine
IROWS = os.environ.get("K_IROWS", "1") == "1"  # interleave rows into mul phase
PREPOFF = os.environ.get("K_PREPOFF", "1") == "1"  # w-prep lo/ul on gpsimd
ROWBUFS = int(os.environ.get("K_ROWBUFS", "4"))
UBUFS = int(os.environ.get("K_UBUFS", "1"))
WLO = os.environ.get("K_WLO", "1") == "1"
LOOP_R = int(os.environ.get("KERNEL_LOOP", "1"))
STAGE = os.environ.get("KERNEL_STAGE", "full")
NBATCH = int(os.environ.get("K_NBATCH", str(B)))

# spread direct-DVE chunks through the 16 so ACT/DVE interleave
_DIRECT = set()
if MULDVE > 0:
    _DIRECT = {int(round(i * NCHUNK / MULDVE)) % NCHUNK for i in range(MULDVE)}


def _build_nc() -> bass.Bass:
    nc = bacc.Bacc()

    # dd8 bytes are fp8e4, but DMA'd as fp16 pairs: the 1-byte-dtype DMA
    # path is ~10x slower on this toolchain; the matmul reads a bitcast view.
    dd8 = nc.declare_dram_parameter("dd8", [B, NSPLIT, M, 2, LH // 2], F16, isOutput=False)
    attnT = nc.declare_dram_parameter("attnT", [P, B * D], F16, isOutput=False)
    embT = nc.declare_dram_parameter("embT", [D, L], F16, isOutput=False)
    vw = nc.declare_dram_parameter("vw", [P, 1], F32, isOutput=False)
    # outp[j, (b*NQ + q)*MMF + f] = out[b, (q*NBANKQ + j)*MMF + f]
    outp = nc.declare_dram_parameter("outp", [NBANKQ, B * NQ * MMF], F16, isOutput=True)

    with TileContext(nc) as tc, ExitStack() as ctx:
        const = ctx.enter_context(tc.tile_pool(name="const", bufs=1))

        vw_sb = const.tile([P, 1], F32)
        nc.scalar.dma_start(out=vw_sb[:], in_=vw[:])
        vw16_sb = const.tile([P, 5], F16)
        for i in range(5):
            nc.scalar.copy(vw16_sb[:, i : i + 1], vw_sb[:])
        attnT_sb = const.tile([P, B * D], F16)
        nc.scalar.dma_start(out=attnT_sb[:], in_=attnT[:])
        embT_sb = const.tile([D, L], F16)
        nc.scalar.dma_start(out=embT_sb[:], in_=embT[:])
        ones32 = const.tile([D, 32], F16)
        nc.vector.memset(ones32[:], 1.0)
        ones101 = const.tile([P, 1], F8)
        nc.vector.memset(ones101[:], 1.0)
        out_sb = const.tile([D, B * NQ * MMF], F16)

        dd_pool = ctx.enter_context(tc.tile_pool(name="dd", bufs=DDBUFS * NSPLIT))
        # pre-write the constant row (two e4m3 1.0 bytes per fp16 slot) in
        # every rotating buffer; in-loop DMAs only touch rows 0..M-1 and the
        # loop allocates a multiple of the buf count per iteration, so the
        # rotation stays aligned and these rows persist.
        # engine APs need base partition % 32 == 0: memset rows 96..100 (the
        # in-loop DMAs overwrite 96..99 with data; row 100 = the const row)
        for _ in range(DDBUFS * NSPLIT):
            t = dd_pool.tile([P, 2, LH // 2], F16, tag="dd")
            nc.vector.memset(t[96:P, :, :], CONST16)
        w_pool = ctx.enter_context(tc.tile_pool(name="w", bufs=2))
        u_psum = ctx.enter_context(tc.tile_pool(name="u", bufs=UBUFS, space="PSUM"))
        g_psum = ctx.enter_context(tc.tile_pool(name="g", bufs=GBUFS, space="PSUM"))
        gs_pool = ctx.enter_context(tc.tile_pool(name="gs", bufs=3))
        prod_pool = ctx.enter_context(tc.tile_pool(name="prod", bufs=2))
        row_psum = ctx.enter_context(tc.tile_pool(name="row", bufs=ROWBUFS, space="PSUM"))

        def emit_rows_q(b, prod, q):
            row = row_psum.tile([D, MMF], F32, tag="row")
            for j in range(NBANKQ):
                h = q * NBANKQ + j
                lsl = slice(h * MMF, (h + 1) * MMF)
                nc.tensor.matmul(
                    row[32 * j : 32 * j + 32, :],
                    lhsT=ones32[:],
                    rhs=prod[:, lsl],
                    start=True,
                    stop=True,
                    tile_position=(0, 32 * j),
                )
            on_dve = (q % 2 == 0) if ROWALT else ROWDVE
            osl = out_sb[:, (b * NQ + q) * MMF : (b * NQ + q + 1) * MMF]
            if on_dve:
                nc.vector.tensor_scalar(
                    out=osl, in0=row[:], scalar1=0.0, scalar2=None,
                    op0=mybir.AluOpType.add,
                )
            else:
                nc.scalar.copy(osl, row[:])

        def emit_rows(b, prod):
            for q in range(NQ):
                emit_rows_q(b, prod, q)

        loop_ctx = tc.For_i(0, LOOP_R, 1) if LOOP_R > 1 else None
        if loop_ctx is not None:
            ctx.enter_context(loop_ctx)
        pending = []
        for b in range(NBATCH):
            eng = nc.gpsimd if RING == "g" else nc.sync
            subs = []
            for s in range(NSPLIT):
                t = dd_pool.tile([P, 2, LH // 2], F16, tag="dd")
                eng.dma_start(out=t[0:M], in_=dd8[b, s])
                subs.append(t[:].bitcast(F8))
            if STAGE == "dma":
                continue

            # weights: w16 = vw*attnT (fp16, d-reversed); hi slots = e4m3(w16);
            # lo slots = e4m3(w16 - hi).  Row CROW: w16 = 0 (vw[CROW]=0) so the
            # lo tile's CROW slots are 0; the hi tile's CROW slots get (uh, ul),
            # an fp8 hi/lo split of the EXACT u = sum_m vw*attn (fp16 matmul),
            # so the const part of ds carries no w-quantization error.
            asl = attnT_sb[:, b * D : (b + 1) * D]
            w16 = w_pool.tile([P, D], F16, tag="w16")
            nc.vector.tensor_scalar(
                out=w16[0:M], in0=asl[0:M], scalar1=vw_sb[0:M, 0:1],
                scalar2=None, op0=mybir.AluOpType.mult,
            )
            # exact u = sum_m vw*attn (fp16 matmul, fp32 accumulate),
            # replicated over psum partitions 96..100 (col base must be %32)
            u_t = u_psum.tile([D, MMF], F32)
            nc.tensor.matmul(
                u_t[96:P, 0:D],
                lhsT=vw16_sb[:],
                rhs=asl,
                start=True,
                stop=True,
                tile_position=(0, 96),
            )
            w8 = w_pool.tile([P, 256], F8, tag="w8")
            w8mm = w8[:].rearrange("p (a b) -> p a b", a=2)
            wv = w8[:].rearrange("p (c j) -> p c j", j=2)
            # const-row weights (uh, ul) land on rows 96..100 FIRST; the data
            # writes below then overwrite rows 96..99, leaving row 100 = (uh, ul)
            nc.scalar.copy(wv[96:P, :, 0], u_t[96:P, 0:D])
            nc.vector.tensor_tensor(
                out=wv[96:P, :, 1],
                in0=u_t[96:P, 0:D],
                in1=wv[96:P, :, 0],
                op=mybir.AluOpType.subtract,
            )
            weng = nc.gpsimd if PREPOFF else nc.vector
            for j in range(2):
                nc.vector.tensor_scalar(
                    out=wv[0:M, :, j], in0=asl[0:M], scalar1=vw_sb[0:M, 0:1],
                    scalar2=None, op0=mybir.AluOpType.mult,
                )
            if WLO:
                wlo8 = w_pool.tile([P, 256], F8, tag="wlo8")
                wlo8mm = wlo8[:].rearrange("p (a b) -> p a b", a=2)
                wlov = wlo8[:].rearrange("p (c j) -> p c j", j=2)
                weng.memset(wlo8[96:P, :], 0.0)
                for j in range(2):
                    weng.tensor_tensor(
                        out=wlov[0:M, :, j], in0=w16[0:M], in1=wv[0:M, :, 0],
                        op=mybir.AluOpType.subtract,
                    )

            prod = prod_pool.tile([D, L], F16)
            for h in range(NCHUNK):
                lsl = slice(h * MMF, (h + 1) * MMF)
                sub = subs[h // CPS]
                c = h % CPS
                g = g_psum.tile([D, MMF], F32)
                rsl = sub[:, :, c * MMF : (c + 1) * MMF]
                nc.tensor.matmul(
                    g[:], lhsT=w8mm, rhs=rsl,
                    start=True, stop=not WLO, perf_mode=DRSW,
                )
                if WLO:
                    nc.tensor.matmul(
                        g[:], lhsT=wlo8mm, rhs=rsl,
                        start=False, stop=True, perf_mode=DRSW,
                    )
                if STAGE == "mm":
                    continue
                if h in _DIRECT:
                    nc.vector.tensor_mul(prod[:, lsl], g[:], embT_sb[:, lsl])
                else:
                    gs = gs_pool.tile([D, MMF], F16)
                    nc.scalar.copy(gs[:], g[:])
                    nc.vector.tensor_mul(prod[:, lsl], gs[:], embT_sb[:, lsl])
                if (
                    STAGE == "full"
                    and IROWS
                    and pending
                    and (h + 1) % NBANKQ == 0
                ):
                    # previous batch's row-quarter rides the PE queue right
                    # after this chunk: deps long satisfied, keeps PE warm
                    pb, pprod = pending[0]
                    emit_rows_q(pb, pprod, h // NBANKQ)
            if STAGE in ("mm", "mul"):
                continue

            # defer rows one batch so deps precede them in every engine FIFO
            if IROWS and pending:
                pending.pop(0)
            pending.append((b, prod))
            if not IROWS and len(pending) > 1:
                emit_rows(*pending.pop(0))
        for bp in pending:
            emit_rows(*bp)

        if STAGE == "full":
            for j in range(NBANKQ):
                nc.scalar.dma_start(
                    out=outp[j], in_=out_sb[32 * j : 32 * j + 1, :]
                )

    nc.compile()
    return nc


_NC_CACHE: dict[str, bass.Bass] = {}


def _get_nc() -> bass.Bass:
    key = (
        f"{MULDVE}:{DDBUFS}:{GBUFS}:{LOOP_R}:{STAGE}:"
        f"{RING}:{NBATCH}:{NSPLIT}:{ROWDVE}:{WLO}:{ROWALT}:{IROWS}:{ROWBUFS}:{UBUFS}:{PREPOFF}"
    )
    if key not in _NC_CACHE:
        _NC_CACHE[key] = _build_nc()
    return _NC_CACHE[key]


def make_in_maps(self_attn, self_delta, emb_table, value_w):
    self_attn = np.asarray(self_attn, dtype=np.float32)
    self_delta = np.asarray(self_delta, dtype=np.float32)
    emb_table = np.asarray(emb_table, dtype=np.float32)
    value_w = np.asarray(value_w, dtype=np.float32).reshape(M)
    assert self_attn.shape == (N, M, D), self_attn.shape
    assert self_delta.shape == (N, M, L, K), self_delta.shape
    assert emb_table.shape == (L + 1, D), emb_table.shape

    # fp8 of (delta - 0.5), laid out [N, NSPLIT, M, 2, LH] (data rows only;
    # the const row lives in SBUF, pre-memset per buffer)
    r8 = (self_delta - np.float32(0.5)).astype(E4)  # [N, M, L, K]
    dd8 = np.ascontiguousarray(
        r8.reshape(N, M, NSPLIT, LH, K).transpose(0, 2, 1, 4, 3)
    )  # [N, NSPLIT, M, K, LH]

    # attnT [P, B*D] fp16, d-reversed, row CROW zero; vw [P,1] with CROW=0
    a = self_attn[:, :, ::-1]  # [N, M, D] reversed d
    attnT_full = np.zeros((N, P, D), dtype=np.float16)
    attnT_full[:, :M] = a
    vw_p = np.zeros((P, 1), dtype=np.float32)
    vw_p[:M, 0] = value_w

    embT16 = np.ascontiguousarray(emb_table[1:].T.astype(np.float16))

    in_maps = []
    for c in range(NCORES):
        nsl = slice(c * B, (c + 1) * B)
        in_maps.append({
            "dd8": np.ascontiguousarray(dd8[nsl]).view(np.float16),
            "attnT": np.ascontiguousarray(
                attnT_full[nsl].transpose(1, 0, 2)
            ).reshape(P, B * D),
            "embT": embT16,
            "vw": vw_p,
        })
    return in_maps


def decode_out(res_outp: np.ndarray) -> np.ndarray:
    """[NBANKQ, B*NQ*MMF] staged fp16 layout -> [B, L] fp32 per core."""
    return (
        res_outp.reshape(NBANKQ, B, NQ, MMF)
        .transpose(1, 2, 0, 3)
        .reshape(B, L)
        .astype(np.float32)
    )


def kernel(self_attn, self_delta, emb_table, value_w, traj_len=None, loc_max=None):
    """Full inputs in, full output out.  traj_len is unused by the reference."""
    if loc_max is not None:
        assert int(loc_max) == L, loc_max
    in_maps = make_in_maps(self_attn, self_delta, emb_table, value_w)

    nc = _get_nc()
    try:
        res = run_bass_kernel_spmd(nc, in_maps, list(range(NCORES)))
    except Exception:
        res = run_bass_kernel_spmd(nc, in_maps, list(range(NCORES)))
    out = np.concatenate(
        [decode_out(res.results[c]["outp"]) for c in range(NCORES)], axis=0
    )
    return out


# revision 12
# speedup vs baseline: 1.3978x; 1.3538x over previous
"""Trainium2 Bass kernel for the fused candidate-attention module (fp8 DRSW).

Computation (reference, fp32):
    ds[n,m,l]  = self_delta[n,m,l,0] + self_delta[n,m,l,1]
    out[n,l]   = sum_m value_w[m] * ds[n,m,l] * (emb[1+l,:] . self_attn[n,m,:])

Sharding: data-parallel over N across 8 cores, B = 4 batches/core, full L.

Quantization scheme (fits the 2e-2 rel-err budget at ~1.3%):
    delta is shipped as fp8e4 of (delta - 0.5); the mul-phase matmul uses
    DoubleRowSwInterleave (DRSW) so BOTH k-planes contract in ONE pass at
    0.5 cyc/row with a free weight load (~112 ns per 512-chunk vs 766 ns
    for the fp16 2-matmul baseline).  The +0.5+0.5 offset is restored
    exactly via an extra contraction row: partition CROW of the moving
    tile is a host-packed constant 1.0, and the matching weight slots hold
    (uh, ul) = an fp8 hi/lo split of u[d] = sum_m w8[m,d], computed on
    device from the *quantized* weights, so the constant part of ds
    carries no w-quantization error.

DRSW layouts (decoded by probe_drsw3.py, exact):
    weights tile [P, 2, 128] flat bytes: byte (p, 2c+j) = w_j[p, 127-c]
      (pairs interleaved, d reversed; host pre-reverses attnT so the
       reversal cancels everywhere on device)
    moving AP [P, 2, 512]: dim1 = the two k-planes as strided blocks
    out[d, f] = sum_p w_0[p,d]*m_0[p,f] + w_1[p,d]*m_1[p,f]

Per-core pipeline per batch: NSPLIT sub-DMAs of [P, 2, LH] fp8 (sync ring);
w8 prep on DVE (vw*attnT into both j slots); u row via a 1-col matmul at
tile_position (0, CROW); 16 DRSW chunk matmuls into PSUM; evict/mul split
between ACT(+DVE) and direct-DVE per K_MULDVE; rows via ones32 col-group
matmuls packed 4/bank, deferred one batch; fp16 out staging.

Env knobs: K_NSPLIT, K_MULDVE (direct-DVE chunks per 16), K_DDBUFS,
K_GBUFS, K_RING, K_ROWDVE (row evictions on DVE instead of ACT),
K_WLO (second w_lo DRSW pass, default 1; halves the w-quant error),
KERNEL_STAGE=dma|mm|mul|full, KERNEL_LOOP=<R>, K_NBATCH.
"""

import os
from contextlib import ExitStack

import numpy as np
import ml_dtypes

import concourse.bacc as bacc
import concourse.bass as bass
import concourse.mybir as mybir
from concourse.bass_utils import run_bass_kernel_spmd
from concourse.tile import TileContext

N, M, L, K, D = 32, 100, 8192, 2, 128
NCORES = 8
B = N // NCORES
MMF = 512
NCHUNK = L // MMF  # 16
NBANKQ = 4
NQ = NCHUNK // NBANKQ

P = 101          # 100 data rows + 1 constant row
CROW = 100       # partition of the constant/offset row
# fp16 value whose two bytes are both e4m3 1.0 (0x38): the const row is
# memset once per dd buffer through the fp16 view
CONST16 = 0.52734375

F32 = mybir.dt.float32
F16 = mybir.dt.float16
F8 = mybir.dt.float8e4
E4 = ml_dtypes.float8_e4m3
DRSW = mybir.MatmulPerfMode.DoubleRowSwInterleave

NSPLIT = int(os.environ.get("K_NSPLIT", "4"))
LH = L // NSPLIT
CPS = NCHUNK // NSPLIT  # chunks per split
MULDVE = int(os.environ.get("K_MULDVE", "4"))
DDBUFS = int(os.environ.get("K_DDBUFS", "2"))
GBUFS = int(os.environ.get("K_GBUFS", "3"))
RING = os.environ.get("K_RING", "1")
ROWDVE = os.environ.get("K_ROWDVE", "0") == "1"
ROWALT = os.environ.get("K_ROWALT", "1") == "1"  # alternate row evict engine
IROWS = os.environ.get("K_IROWS", "1") == "1"  # interleave rows into mul phase
PREPOFF = os.environ.get("K_PREPOFF", "1") == "1"  # w-prep lo/ul on gpsimd
ROWBUFS = int(os.environ.get("K_ROWBUFS", "4"))
UBUFS = int(os.environ.get("K_UBUFS", "1"))
WLO = os.environ.get("K_WLO", "1") == "1"
LOOP_R = int(os.environ.get("KERNEL_LOOP", "1"))
STAGE = os.environ.get("KERNEL_STAGE", "full")
NBATCH = int(os.environ.get("K_NBATCH", str(B)))

# spread direct-DVE chunks through the 16 so ACT/DVE interleave
_DIRECT = set()
if MULDVE > 0:
    _DIRECT = {int(round(i * NCHUNK / MULDVE)) % NCHUNK for i in range(MULDVE)}


def _build_nc() -> bass.Bass:
    nc = bacc.Bacc()

    # dd8 bytes are fp8e4, but DMA'd as fp16 pairs: the 1-byte-dtype DMA
    # path is ~10x slower on this toolchain; the matmul reads a bitcast view.
    dd8 = nc.declare_dram_parameter("dd8", [B, NSPLIT, M, 2, LH // 2], F16, isOutput=False)
    attnT = nc.declare_dram_parameter("attnT", [P, B * D], F16, isOutput=False)
    embT = nc.declare_dram_parameter("embT", [D, L], F16, isOutput=False)
    vw = nc.declare_dram_parameter("vw", [P, 1], F32, isOutput=False)
    cst = nc.declare_dram_parameter("cst", [5, 2 * (LH // 2)], F16, isOutput=False)
    # outp[j, (b*NQ + q)*MMF + f] = out[b, (q*NBANKQ + j)*MMF + f]
    outp = nc.declare_dram_parameter("outp", [NBANKQ, B * NQ * MMF], F16, isOutput=True)

    with TileContext(nc) as tc, ExitStack() as ctx:
        const = ctx.enter_context(tc.tile_pool(name="const", bufs=1))

        vw_sb = const.tile([P, 1], F32)
        nc.scalar.dma_start(out=vw_sb[:], in_=vw[:])
        vw16_sb = const.tile([P, 5], F16)
        for i in range(5):
            nc.scalar.copy(vw16_sb[:, i : i + 1], vw_sb[:])
        attnT_sb = const.tile([P, B * D], F16)
        nc.scalar.dma_start(out=attnT_sb[:], in_=attnT[:])
        embT_sb = const.tile([D, L], F16)
        nc.scalar.dma_start(out=embT_sb[:], in_=embT[:])
        ones32 = const.tile([D, 32], F16)
        nc.vector.memset(ones32[:], 1.0)
        ones101 = const.tile([P, 1], F8)
        nc.vector.memset(ones101[:], 1.0)
        out_sb = const.tile([D, B * NQ * MMF], F16)

        dd_pool = ctx.enter_context(tc.tile_pool(name="dd", bufs=DDBUFS * NSPLIT))
        # pre-write the constant row (two e4m3 1.0 bytes per fp16 slot) in
        # every rotating buffer; in-loop DMAs only touch rows 0..M-1 and the
        # loop allocates a multiple of the buf count per iteration, so the
        # rotation stays aligned and these rows persist.
        # const row 100 of each rotating buffer, loaded once at setup from a
        # tiny DRAM blob (rows 96..99 are scratch; in-loop DMAs overwrite them)
        for _ in range(DDBUFS * NSPLIT):
            t = dd_pool.tile([P, 2, LH // 2], F16, tag="dd")
            nc.gpsimd.dma_start(
                out=t[96:P, :, :].rearrange("p a b -> p (a b)"), in_=cst[:]
            )
        w_pool = ctx.enter_context(tc.tile_pool(name="w", bufs=2))
        u_psum = ctx.enter_context(tc.tile_pool(name="u", bufs=UBUFS, space="PSUM"))
        g_psum = ctx.enter_context(tc.tile_pool(name="g", bufs=GBUFS, space="PSUM"))
        gs_pool = ctx.enter_context(tc.tile_pool(name="gs", bufs=3))
        prod_pool = ctx.enter_context(tc.tile_pool(name="prod", bufs=2))
        row_psum = ctx.enter_context(tc.tile_pool(name="row", bufs=ROWBUFS, space="PSUM"))

        def emit_rows_q(b, prod, q):
            row = row_psum.tile([D, MMF], F32, tag="row")
            for j in range(NBANKQ):
                h = q * NBANKQ + j
                lsl = slice(h * MMF, (h + 1) * MMF)
                nc.tensor.matmul(
                    row[32 * j : 32 * j + 32, :],
                    lhsT=ones32[:],
                    rhs=prod[:, lsl],
                    start=True,
                    stop=True,
                    tile_position=(0, 32 * j),
                )
            on_dve = (q % 2 == 0) if ROWALT else ROWDVE
            osl = out_sb[:, (b * NQ + q) * MMF : (b * NQ + q + 1) * MMF]
            if on_dve:
                nc.vector.tensor_scalar(
                    out=osl, in0=row[:], scalar1=0.0, scalar2=None,
                    op0=mybir.AluOpType.add,
                )
            else:
                nc.scalar.copy(osl, row[:])

        def emit_rows(b, prod):
            for q in range(NQ):
                emit_rows_q(b, prod, q)

        loop_ctx = tc.For_i(0, LOOP_R, 1) if LOOP_R > 1 else None
        if loop_ctx is not None:
            ctx.enter_context(loop_ctx)
        pending = []
        for b in range(NBATCH):
            eng = nc.gpsimd if RING == "g" else nc.sync
            subs = []
            for s in range(NSPLIT):
                t = dd_pool.tile([P, 2, LH // 2], F16, tag="dd")
                eng.dma_start(out=t[0:M], in_=dd8[b, s])
                subs.append(t[:].bitcast(F8))
            if STAGE == "dma":
                continue

            # weights: w16 = vw*attnT (fp16, d-reversed); hi slots = e4m3(w16);
            # lo slots = e4m3(w16 - hi).  Row CROW: w16 = 0 (vw[CROW]=0) so the
            # lo tile's CROW slots are 0; the hi tile's CROW slots get (uh, ul),
            # an fp8 hi/lo split of the EXACT u = sum_m vw*attn (fp16 matmul),
            # so the const part of ds carries no w-quantization error.
            asl = attnT_sb[:, b * D : (b + 1) * D]
            w16 = w_pool.tile([P, D], F16, tag="w16")
            nc.vector.tensor_scalar(
                out=w16[0:M], in0=asl[0:M], scalar1=vw_sb[0:M, 0:1],
                scalar2=None, op0=mybir.AluOpType.mult,
            )
            # exact u = sum_m vw*attn (fp16 matmul, fp32 accumulate),
            # replicated over psum partitions 96..100 (col base must be %32)
            u_t = u_psum.tile([D, MMF], F32)
            nc.tensor.matmul(
                u_t[96:P, 0:D],
                lhsT=vw16_sb[:],
                rhs=asl,
                start=True,
                stop=True,
                tile_position=(0, 96),
            )
            w8 = w_pool.tile([P, 256], F8, tag="w8")
            w8mm = w8[:].rearrange("p (a b) -> p a b", a=2)
            wv = w8[:].rearrange("p (c j) -> p c j", j=2)
            # const-row weights (uh, ul) land on rows 96..100 FIRST; the data
            # writes below then overwrite rows 96..99, leaving row 100 = (uh, ul)
            nc.scalar.copy(wv[96:P, :, 0], u_t[96:P, 0:D])
            nc.vector.tensor_tensor(
                out=wv[96:P, :, 1],
                in0=u_t[96:P, 0:D],
                in1=wv[96:P, :, 0],
                op=mybir.AluOpType.subtract,
            )
            weng = nc.gpsimd if PREPOFF else nc.vector
            for j in range(2):
                nc.vector.tensor_scalar(
                    out=wv[0:M, :, j], in0=asl[0:M], scalar1=vw_sb[0:M, 0:1],
                    scalar2=None, op0=mybir.AluOpType.mult,
                )
            if WLO:
                wlo8 = w_pool.tile([P, 256], F8, tag="wlo8")
                wlo8mm = wlo8[:].rearrange("p (a b) -> p a b", a=2)
                wlov = wlo8[:].rearrange("p (c j) -> p c j", j=2)
                weng.memset(wlo8[96:P, :], 0.0)
                for j in range(2):
                    weng.tensor_tensor(
                        out=wlov[0:M, :, j], in0=w16[0:M], in1=wv[0:M, :, 0],
                        op=mybir.AluOpType.subtract,
                    )

            prod = prod_pool.tile([D, L], F16)
            for h in range(NCHUNK):
                lsl = slice(h * MMF, (h + 1) * MMF)
                sub = subs[h // CPS]
                c = h % CPS
                g = g_psum.tile([D, MMF], F32)
                rsl = sub[:, :, c * MMF : (c + 1) * MMF]
                nc.tensor.matmul(
                    g[:], lhsT=w8mm, rhs=rsl,
                    start=True, stop=not WLO, perf_mode=DRSW,
                )
                if WLO:
                    nc.tensor.matmul(
                        g[:], lhsT=wlo8mm, rhs=rsl,
                        start=False, stop=True, perf_mode=DRSW,
                    )
                if STAGE == "mm":
                    continue
                if h in _DIRECT:
                    nc.vector.tensor_mul(prod[:, lsl], g[:], embT_sb[:, lsl])
                else:
                    gs = gs_pool.tile([D, MMF], F16)
                    nc.scalar.copy(gs[:], g[:])
                    nc.vector.tensor_mul(prod[:, lsl], gs[:], embT_sb[:, lsl])
                if (
                    STAGE == "full"
                    and IROWS
                    and pending
                    and (h + 1) % NBANKQ == 0
                ):
                    # previous batch's row-quarter rides the PE queue right
                    # after this chunk: deps long satisfied, keeps PE warm
                    pb, pprod = pending[0]
                    emit_rows_q(pb, pprod, h // NBANKQ)
            if STAGE in ("mm", "mul"):
                continue

            # defer rows one batch so deps precede them in every engine FIFO
            if IROWS and pending:
                pending.pop(0)
            pending.append((b, prod))
            if not IROWS and len(pending) > 1:
                emit_rows(*pending.pop(0))
        for bp in pending:
            emit_rows(*bp)

        if STAGE == "full":
            for j in range(NBANKQ):
                nc.scalar.dma_start(
                    out=outp[j], in_=out_sb[32 * j : 32 * j + 1, :]
                )

    nc.compile()
    return nc


_NC_CACHE: dict[str, bass.Bass] = {}


def _get_nc() -> bass.Bass:
    key = (
        f"{MULDVE}:{DDBUFS}:{GBUFS}:{LOOP_R}:{STAGE}:"
        f"{RING}:{NBATCH}:{NSPLIT}:{ROWDVE}:{WLO}:{ROWALT}:{IROWS}:{ROWBUFS}:{UBUFS}:{PREPOFF}"
    )
    if key not in _NC_CACHE:
        _NC_CACHE[key] = _build_nc()
    return _NC_CACHE[key]


def make_in_maps(self_attn, self_delta, emb_table, value_w):
    self_attn = np.asarray(self_attn, dtype=np.float32)
    self_delta = np.asarray(self_delta, dtype=np.float32)
    emb_table = np.asarray(emb_table, dtype=np.float32)
    value_w = np.asarray(value_w, dtype=np.float32).reshape(M)
    assert self_attn.shape == (N, M, D), self_attn.shape
    assert self_delta.shape == (N, M, L, K), self_delta.shape
    assert emb_table.shape == (L + 1, D), emb_table.shape

    # fp8 of (delta - 0.5), laid out [N, NSPLIT, M, 2, LH] (data rows only;
    # the const row lives in SBUF, pre-memset per buffer)
    r8 = (self_delta - np.float32(0.5)).astype(E4)  # [N, M, L, K]
    dd8 = np.ascontiguousarray(
        r8.reshape(N, M, NSPLIT, LH, K).transpose(0, 2, 1, 4, 3)
    )  # [N, NSPLIT, M, K, LH]

    # attnT [P, B*D] fp16, d-reversed, row CROW zero; vw [P,1] with CROW=0
    a = self_attn[:, :, ::-1]  # [N, M, D] reversed d
    attnT_full = np.zeros((N, P, D), dtype=np.float16)
    attnT_full[:, :M] = a
    vw_p = np.zeros((P, 1), dtype=np.float32)
    vw_p[:M, 0] = value_w

    embT16 = np.ascontiguousarray(emb_table[1:].T.astype(np.float16))
    cst = np.full((5, LH), CONST16, dtype=np.float16)

    in_maps = []
    for c in range(NCORES):
        nsl = slice(c * B, (c + 1) * B)
        in_maps.append({
            "dd8": np.ascontiguousarray(dd8[nsl]).view(np.float16),
            "attnT": np.ascontiguousarray(
                attnT_full[nsl].transpose(1, 0, 2)
            ).reshape(P, B * D),
            "embT": embT16,
            "vw": vw_p,
            "cst": cst,
        })
    return in_maps


def decode_out(res_outp: np.ndarray) -> np.ndarray:
    """[NBANKQ, B*NQ*MMF] staged fp16 layout -> [B, L] fp32 per core."""
    return (
        res_outp.reshape(NBANKQ, B, NQ, MMF)
        .transpose(1, 2, 0, 3)
        .reshape(B, L)
        .astype(np.float32)
    )


def kernel(self_attn, self_delta, emb_table, value_w, traj_len=None, loc_max=None):
    """Full inputs in, full output out.  traj_len is unused by the reference."""
    if loc_max is not None:
        assert int(loc_max) == L, loc_max
    in_maps = make_in_maps(self_attn, self_delta, emb_table, value_w)

    nc = _get_nc()
    try:
        res = run_bass_kernel_spmd(nc, in_maps, list(range(NCORES)))
    except Exception:
        res = run_bass_kernel_spmd(nc, in_maps, list(range(NCORES)))
    out = np.concatenate(
        [decode_out(res.results[c]["outp"]) for c in range(NCORES)], axis=0
    )
    return out
